# revision 26
# baseline (speedup 1.0000x reference)
"""Trainium2 Bass kernel for nn_LonelyDecoder (dense transformer, 8-core TP).

Key observations baked in:
 - In the reference, every layer recomputes from the embedding output `h`
   and only the LAST layer's `out` feeds the logits -> layers 0..L-2 are
   dead code. We compute: embedding GEMM, layer L-1, output GEMM+softmax.
 - Scores are tiny ((q.k)/1024, |s| < ~1), so softmax needs no max pass.
 - All activations are kept TRANSPOSED ([feature, seq]) so that:
     * matmul contraction dims land on partitions for both operands,
     * per-feature biases are per-partition (fused into ACT evictions),
     * head-concat == AllGather along the partition axis.

Sharding (8 cores):
 - vocab dim of x/emb_W/out_W (4000/core, padded to 4096)
 - heads of attention (2 heads/core), DFF of the FFN (512/core)

Schedule (v2, vs the v1 baseline):
 - embedding AllReduce chunked per 512-col s-chunk, overlapped with the
   GEMM of later chunks; hT prep per chunk.
 - attention AllGathers chunked per s-chunk; layernorm restructured
   per-s-chunk so it overlaps the next phase.
 - FFN AllReduce in bf16, chunked per s-chunk, overlapped with FFN
   compute and the ln3/output-GEMM head.
 - output GEMM loops vocab-chunk outer with out_W streamed; exp() tiles
   stay resident in SBUF so the softmax tail is just (mul by 1/sum) +
   writeout, no DRAM round trip.
 - scores exp() batched over pairs of t-tiles ([128,1024] ACTs).
 - reciprocals via the fast custom-DVE approximation (~5x faster).
"""

import numpy as np
import ml_dtypes

import concourse.bacc as bacc
import concourse.bass as bass
import concourse.mybir as mybir
import concourse.tile as tile
from concourse.bass_utils import run_bass_kernel_spmd

F32 = mybir.dt.float32
F16 = mybir.dt.float16
BF16 = mybir.dt.bfloat16
AF = mybir.ActivationFunctionType
ALU = mybir.AluOpType

S, V, D, H, DK, DFF, L = 2048, 32000, 1024, 16, 64, 4096, 4
NCORES = 8
VSR = V // NCORES          # 4000 real vocab shard
VSP = 4096                 # padded vocab shard (32 x 128)
NVC = VSP // 128           # 32 v-chunks
NDC = D // 128             # 8 d-chunks
NSC = 4                    # s-chunks of 512
SC = 512
NTT = S // 128             # 16 t-tiles
FS = DFF // NCORES         # 512 ff shard
NFC = FS // 128            # 4 ff chunks
RG = [list(range(NCORES))]

LAST_RESULTS = {}          # stash for test harness (exec time etc.)


def ts(i, n):
    return slice(i * n, (i + 1) * n)


def build_bass(debug=False):
    nc = bacc.Bacc(None, target_bir_lowering=False)

    # ---- I/O ----
    xT = nc.dram_tensor("xT", [VSP, S], BF16, kind="ExternalInput")
    embW = nc.dram_tensor("embW", [VSP, D], BF16, kind="ExternalInput")
    pebT = nc.dram_tensor("pebT", [D, S], BF16, kind="ExternalInput")
    qkw = [nc.dram_tensor(f"qkw{m}", [128, NDC, 256], BF16, kind="ExternalInput") for m in (1, 2)]
    bqk = [nc.dram_tensor(f"bqk{m}", [128, 2], F32, kind="ExternalInput") for m in (1, 2)]
    vw = [nc.dram_tensor(f"vw{m}", [128, NDC, 130], BF16, kind="ExternalInput") for m in (1, 2)]
    bv = [nc.dram_tensor(f"bv{m}", [128, 1], F32, kind="ExternalInput") for m in (1, 2)]
    maskT = nc.dram_tensor("maskT", [128, 4 * SC], BF16, kind="ExternalInput")
    f1w = nc.dram_tensor("f1w", [128, NDC, FS], BF16, kind="ExternalInput")
    f1b = nc.dram_tensor("f1b", [128, NFC], F32, kind="ExternalInput")
    f2w = nc.dram_tensor("f2w", [128, NFC, D], BF16, kind="ExternalInput")
    f2bT = nc.dram_tensor("f2bT", [128, NDC], F32, kind="ExternalInput")
    lngT = nc.dram_tensor("lngT", [128, NDC], F32, kind="ExternalInput")
    lnbT = nc.dram_tensor("lnbT", [128, NDC], F32, kind="ExternalInput")
    outw = nc.dram_tensor("outw", [NVC, 128, NDC, 128], BF16, kind="ExternalInput")
    outb = nc.dram_tensor("outb", [128, NVC], F32, kind="ExternalInput")
    probsT = nc.dram_tensor("probsT", [VSP, S], F16, kind="ExternalOutput")
    dbg = {}
    if debug:
        for nm in ("hT", "h1T", "h2T", "outT"):
            dbg[nm] = nc.dram_tensor(f"dbg_{nm}", [D, S], BF16, kind="ExternalOutput")
        dbg["yr"] = nc.dram_tensor("dbg_yr", [D, S], BF16, kind="ExternalOutput")
        dbg["hpar"] = nc.dram_tensor("dbg_hpar", [D, S], F32, kind="ExternalOutput")
        dbg["q0"] = nc.dram_tensor("dbg_q0", [128, S], BF16, kind="ExternalOutput")
        dbg["k0"] = nc.dram_tensor("dbg_k0", [128, S], BF16, kind="ExternalOutput")
        dbg["V0"] = nc.dram_tensor("dbg_V0", [128, NTT * 130], BF16, kind="ExternalOutput")
        dbg["ai0"] = nc.dram_tensor("dbg_ai0", [128, S], BF16, kind="ExternalOutput")
        dbg["et0"] = nc.dram_tensor("dbg_et0", [128, SC], BF16, kind="ExternalOutput")
        dbg["po0"] = nc.dram_tensor("dbg_po0", [65, SC], F32, kind="ExternalOutput")
        dbg["oo0"] = nc.dram_tensor("dbg_oo0", [65, SC], F32, kind="ExternalOutput")
        dbg["rec0"] = nc.dram_tensor("dbg_rec0", [1, SC], F32, kind="ExternalOutput")
        dbg["hT0"] = nc.dram_tensor("dbg_hT0", [D, S], BF16, kind="ExternalOutput")
        dbg["a0"] = nc.dram_tensor("dbg_a0", [D, S], BF16, kind="ExternalOutput")

    with tile.TileContext(nc) as tc:
        with tc.tile_pool(name="dram", bufs=1, space="DRAM") as dram, \
             tc.tile_pool(name="const", bufs=1) as const, \
             tc.tile_pool(name="outp", bufs=1) as outp, \
             tc.tile_pool(name="outwp", bufs=3) as owp:

            # internal DRAM (tracked pool tiles)
            h_par = [dram.tile([D, SC], BF16, tag=f"hp{sc}", name=f"h_par{sc}")
                     for sc in range(NSC)]
            h_red = [dram.tile([D, SC], BF16, tag=f"hr{sc}", addr_space="Shared",
                               name=f"h_red{sc}") for sc in range(NSC)]
            a_in = [[dram.tile([128, SC], BF16, tag=f"a{m}i{sc}", name=f"a{m}_in{sc}")
                     for sc in range(NSC)] for m in (0, 1)]
            a_out = [[dram.tile([D, SC], BF16, tag=f"a{m}o{sc}", addr_space="Shared",
                                name=f"a{m}_out{sc}") for sc in range(NSC)]
                     for m in (0, 1)]
            y_par = [dram.tile([D, 2 * SC], BF16, tag=f"yp{sc}", name=f"y_par{sc}")
                     for sc in range(2)]
            y_red = [dram.tile([D, 2 * SC], BF16, tag=f"yr{sc}", addr_space="Shared",
                               name=f"y_red{sc}") for sc in range(2)]
            ss_in = [dram.tile([1, 2 * SC], F32, tag=f"ssi{hf}", name=f"ss_in{hf}")
                     for hf in range(2)]
            ss_out = [dram.tile([1, 2 * SC], F32, tag=f"sso{hf}", addr_space="Shared",
                                name=f"ss_out{hf}") for hf in range(2)]

            # constants
            ones_bf_col = const.tile([128, 1], BF16, tag="c1")
            nc.vector.memset(ones_bf_col[:, :], 1.0)
            ones_row = const.tile([1, 128], F32, tag="c3")
            nc.vector.memset(ones_row[:, :], 1.0)
            ones_row64 = const.tile([1, 64], F32, tag="c4")
            nc.vector.memset(ones_row64[:, :], 1.0)
            eps_tile = const.tile([1, 1], F32, tag="c5")
            nc.vector.memset(eps_tile[:, :], 1e-5)
            bqk_sb = [const.tile([128, 2], F32, tag=f"bqk{m}", name=f"bqk_sb{m}") for m in range(2)]
            bv_sb = [const.tile([128, 1], F32, tag=f"bv{m}", name=f"bv_sb{m}") for m in range(2)]
            for m in range(2):
                nc.sync.dma_start(bqk_sb[m][:, :], bqk[m][:, :])
                nc.sync.dma_start(bv_sb[m][:, :], bv[m][:, :])
            f1b_sb = const.tile([128, NFC], F32, tag="f1b")
            nc.sync.dma_start(f1b_sb[:, :], f1b[:, :])
            f2bT_sb = const.tile([128, NDC], F32, tag="f2bT")
            nc.sync.dma_start(f2bT_sb[:, :], f2bT[:, :])
            lng_sb = const.tile([128, NDC], F32, tag="lng")
            nc.sync.dma_start(lng_sb[:, :], lngT[:, :])
            lnb_sb = const.tile([128, NDC], F32, tag="lnb")
            nc.sync.dma_start(lnb_sb[:, :], lnbT[:, :])
            outb_sb = const.tile([128, NVC], F32, tag="outb")
            nc.sync.dma_start(outb_sb[:, :], outb[:, :])

            acts_ctx = tc.tile_pool(name="acts", bufs=2)
            acts = acts_ctx.__enter__()
            hT = acts.tile([128, NDC, S], BF16, tag="act", name="hT")

            # ---------- embedding GEMM (chunked AllReduce overlap) ----------
            with tc.tile_pool(name="embw", bufs=1) as embp, \
                 tc.tile_pool(name="xt", bufs=3) as xtp, \
                 tc.tile_pool(name="peb", bufs=1) as pebp, \
                 tc.tile_pool(name="ps_e", bufs=1, space="PSUM") as pse, \
                 tc.tile_pool(name="ev_e", bufs=3) as evp, \
                 tc.tile_pool(name="addin_e", bufs=3) as adpe:
                peb_sb = pebp.tile([128, NDC, S], BF16, tag="peb")
                for dc in range(NDC):
                    nc.sync.dma_start(peb_sb[:, dc, :], pebT[ts(dc, 128), :])

                def ht_prep(psc):
                    for dc in range(NDC):
                        hr = adpe.tile([128, SC], BF16, tag="addin",
                                       name=f"hr_{psc}_{dc}")
                        nc.scalar.dma_start(hr[:, :], h_red[psc][ts(dc, 128), :])
                        nc.vector.tensor_add(hT[:, dc, ts(psc, SC)], hr[:, :],
                                             peb_sb[:, dc, ts(psc, SC)])

                for sc in range(NSC):
                    pes = [pse.tile([128, SC], F32, tag=f"pe{dc}", name=f"pe_{sc}_{dc}")
                           for dc in range(NDC)]
                    for kc in range(NVC):
                        xt = xtp.tile([128, SC], BF16, tag="xt")
                        nc.sync.dma_start(xt[:, :], xT[ts(kc, 128), ts(sc, SC)])
                        ew = embp.tile([128, D], BF16, tag="ew", bufs=6,
                                       name=f"ew_{sc}_{kc}")
                        nc.sync.dma_start(ew[:, :], embW[ts(kc, 128), :])
                        for dc in range(NDC):
                            nc.tensor.matmul(
                                pes[dc][:, :],
                                ew[:, ts(dc, 128)],
                                xt[:, :],
                                start=(kc == 0),
                                stop=(kc == NVC - 1),
                            )
                    for dc in range(NDC):
                        hv = evp.tile([128, SC], BF16, tag="ev")
                        nc.scalar.activation(hv[:, :], pes[dc][:, :], AF.Copy)
                        nc.sync.dma_start(h_par[sc][ts(dc, 128), :], hv[:, :])
                    if debug:
                        nc.sync.dma_start(dbg["hpar"][:, ts(sc, SC)], h_par[sc][:, :])
                    nc.gpsimd.collective_compute(
                        "AllReduce", ALU.add, replica_groups=RG,
                        ins=[h_par[sc][:, :].opt()], outs=[h_red[sc][:, :].opt()],
                    )
                    if sc > 0:
                        ht_prep(sc - 1)
                ht_prep(NSC - 1)

            # ======== phase A: attention x2, layernorms, FFN ========
            # PSUM budget (8 banks): ps1(2) + po(2) + pg(2x2=4)
            with tc.tile_pool(name="addin", bufs=4) as adp, \
                 tc.tile_pool(name="x2p", bufs=2) as x2p, \
                 tc.tile_pool(name="ev_a", bufs=3) as evp, \
                 tc.tile_pool(name="small_a", bufs=2) as smp, \
                 tc.tile_pool(name="ps_a", bufs=2, space="PSUM") as psa:

                def mha_block(mi, actT, masked, attnp, qkw_sb, vw_sb):
                    """mi: 0/1 selects weight set; actT: [128, NDC, S] bf16.
                    Per s-chunk: 2 heads' attention rows -> a_in[mi][sc],
                    progressive AllGather into a_out[mi][sc]."""
                    V_sb = attnp.tile([128, NTT, 130], BF16, tag="V", name=f"V_sb{mi}")
                    qT2 = attnp.tile([128, NSC, SC], BF16, tag="qT2", name=f"qT2_{mi}")
                    kT2 = attnp.tile([128, NSC, SC], BF16, tag="kT2", name=f"kT2_{mi}")
                    for sc in range(NSC):
                        # V~ = [V_h0 | 1 | V_h1 | 1]  laid out [t, 130]
                        for tt in range(4 * sc, 4 * sc + 4):
                            pv = psa.tile([128, SC], F32, tag="ps1", name=f"pv{mi}_{tt}")
                            for dc in range(NDC):
                                nc.tensor.matmul(
                                    pv[:, 0:130], actT[:, dc, ts(tt, 128)], vw_sb[:, dc, :],
                                    start=(dc == 0), stop=(dc == NDC - 1),
                                )
                            nc.scalar.activation(V_sb[:, tt, :], pv[:, 0:130], AF.Copy)
                        for wi, dst in ((0, qT2), (1, kT2)):
                            pq = psa.tile([128, SC], F32, tag="ps1", name=f"pq{mi}_{wi}_{sc}")
                            for dc in range(NDC):
                                nc.tensor.matmul(
                                    pq[:, :],
                                    qkw_sb[:, dc, ts(wi, 128)],
                                    actT[:, dc, ts(sc, SC)],
                                    start=(dc == 0), stop=(dc == NDC - 1),
                                )
                            nc.scalar.activation(
                                dst[:, sc, :], pq[:, :], AF.Identity,
                                bias=bqk_sb[mi][:, wi:wi + 1],
                            )
                    nc.vector.memset(V_sb[:, :, 64:65], 1.0)
                    nc.vector.memset(V_sb[:, :, 129:130], 1.0)
                    if debug and mi == 0:
                        nc.sync.dma_start(dbg["q0"][:, :], qT2[:, :, :])
                        nc.sync.dma_start(dbg["k0"][:, :], kT2[:, :, :])
                        nc.sync.dma_start(dbg["V0"][:, :], V_sb[:, :, :])
                        for dc in range(NDC):
                            nc.sync.dma_start(dbg["hT0"][ts(dc, 128), :], actT[:, dc, :])

                    attnT = attnp.tile([128, NSC, SC], BF16, tag="attnT", name=f"attnT{mi}")
                    for sc in range(NSC):
                        for h in range(2):
                            po = psa.tile([128, SC], F32, tag="po", name=f"po{mi}_{h}_{sc}")
                            tts = list(range(4 * (sc + 1))) if masked else list(range(NTT))
                            pairs = [tts[i:i + 2] for i in range(0, len(tts), 2)]
                            for pi, pr in enumerate(pairs):
                                pg = psa.tile([128, 2 * SC], F32, tag="pg",
                                              name=f"pg{mi}_{h}_{sc}_{pi}")
                                for j, tt in enumerate(pr):
                                    nc.tensor.matmul(
                                        pg[:, ts(j, SC)],
                                        kT2[ts(h, 64), tt // 4, ts(tt % 4, 128)],
                                        qT2[ts(h, 64), sc, :],
                                        start=True, stop=True,
                                    )
                                et = evp.tile([128, 2 * SC], BF16, tag="exp")
                                nc.scalar.activation(et[:, :], pg[:, :], AF.Exp,
                                                     scale=1.0 / D)
                                if masked and pr[0] >= 4 * sc:
                                    mo = (pr[0] - 4 * sc) * SC
                                    nc.vector.tensor_mul(
                                        et[:, :], et[:, :],
                                        mask_sb[:, mo:mo + 2 * SC],
                                    )
                                for j, tt in enumerate(pr):
                                    nc.tensor.matmul(
                                        po[0:65, :],
                                        V_sb[:, tt, ts(h, 65)],
                                        et[:, ts(j, SC)],
                                        start=(pi == 0 and j == 0),
                                        stop=(pi == len(pairs) - 1 and j == len(pr) - 1),
                                    )
                            oo = smp.tile([64, SC], F32, tag="oo", name=f"oo{mi}_{h}_{sc}")
                            nc.scalar.activation(oo[:, :], po[0:64, :], AF.Copy)
                            s0 = smp.tile([1, SC], F32, tag="s0", name=f"s0{mi}_{h}_{sc}")
                            nc.scalar.activation(s0[:, :], po[64:65, :], AF.Copy)
                            rec = smp.tile([1, SC], F32, tag="rec", name=f"rec{mi}_{h}_{sc}")
                            nc.vector.reciprocal_approx_fast(rec[:, :], s0[:, :])
                            pbv = psa.tile([128, SC], F32, tag="ps1", name=f"pb{mi}_{h}_{sc}")
                            nc.tensor.matmul(pbv[0:64, :], ones_row64[:, :], rec[:, :],
                                             start=True, stop=True)
                            tmp = smp.tile([64, SC], F32, tag="avtmp", name=f"avtmp{mi}_{h}_{sc}")
                            nc.vector.tensor_mul(tmp[:, :], oo[:, :], pbv[0:64, :])
                            nc.scalar.activation(
                                attnT[ts(h, 64), sc, :], tmp[:, :], AF.Identity,
                                bias=bv_sb[mi][ts(h, 64), :],
                            )
                        nc.sync.dma_start(a_in[mi][sc][:, :], attnT[:, sc, :])
                        if debug and mi == 0:
                            nc.sync.dma_start(dbg["ai0"][:, ts(sc, SC)], attnT[:, sc, :])
                        nc.gpsimd.collective_compute(
                            "AllGather", ALU.bypass, replica_groups=RG,
                            ins=[a_in[mi][sc][:, :].opt()],
                            outs=[a_out[mi][sc][:, :].opt()],
                        )

                # residual + layernorm over feature dim, per s-chunk.
                # addin_fn(sc, dc) -> (dram_ap, per-partition bias AP or None)
                def ln_block(prevT, addin_fn, name, dst_pool=None, dst_tag="act",
                             ad_eng=None):
                    dp = dst_pool if dst_pool is not None else acts
                    newT = dp.tile([128, NDC, S], BF16, tag=dst_tag, name=name)
                    for sc in range(NSC):
                        stats = psa.tile([65, SC], F32, tag="ps1",
                                         name=f"st_{name}_{sc}")
                        for dc in range(NDC):
                            src_ap, xbias = addin_fn(sc, dc)
                            ad = adp.tile([128, SC], BF16, tag="addin",
                                          name=f"ad_{name}_{sc}_{dc}")
                            (ad_eng or nc.sync).dma_start(ad[:, :], src_ap)
                            sl = ts(sc, SC)
                            if xbias is not None:
                                nc.vector.scalar_tensor_tensor(
                                    prevT[:, dc, sl], ad[:, :], xbias,
                                    prevT[:, dc, sl], op0=ALU.add, op1=ALU.add)
                            else:
                                nc.vector.tensor_add(prevT[:, dc, sl],
                                                     prevT[:, dc, sl], ad[:, :])
                            x2 = x2p.tile([128, SC], BF16, tag="x2",
                                          name=f"x2_{name}_{sc}_{dc}")
                            nc.vector.tensor_mul(x2[:, :], prevT[:, dc, sl],
                                                 prevT[:, dc, sl])
                            nc.tensor.matmul(stats[0:1, :], ones_bf_col[:, :],
                                             prevT[:, dc, sl],
                                             start=(dc == 0), stop=(dc == NDC - 1))
                            nc.tensor.matmul(stats[64:65, :], ones_bf_col[:, :],
                                             x2[:, :],
                                             start=(dc == 0), stop=(dc == NDC - 1))
                        nm = smp.tile([1, SC], F32, tag="nm", name=f"nm_{name}_{sc}")
                        nc.vector.tensor_scalar_mul(nm[:, :], stats[0:1, :], -1.0 / D)
                        e2 = smp.tile([1, SC], F32, tag="e2", name=f"e2_{name}_{sc}")
                        nc.vector.tensor_scalar_mul(e2[:, :], stats[64:65, :], 1.0 / D)
                        musq = smp.tile([1, SC], F32, tag="musq", name=f"musq_{name}_{sc}")
                        nc.vector.tensor_mul(musq[:, :], nm[:, :], nm[:, :])
                        nc.vector.tensor_sub(e2[:, :], e2[:, :], musq[:, :])
                        sd = smp.tile([1, SC], F32, tag="sd", name=f"sd_{name}_{sc}")
                        nc.scalar.activation(sd[:, :], e2[:, :], AF.Sqrt,
                                             bias=eps_tile[:, :])
                        inv = smp.tile([1, SC], F32, tag="inv", name=f"inv_{name}_{sc}")
                        nc.vector.reciprocal_approx_fast(inv[:, :], sd[:, :])
                        ninv = smp.tile([1, SC], F32, tag="ninv", name=f"ninv_{name}_{sc}")
                        nc.vector.tensor_mul(ninv[:, :], nm[:, :], inv[:, :])
                        # broadcasts share one 2-bank "pg" slot: [inv | ninv]
                        pgx = psa.tile([128, 2 * SC], F32, tag="pg",
                                       name=f"pgx_{name}_{sc}")
                        nc.tensor.matmul(pgx[:, 0:SC], ones_row[:, :], inv[:, :],
                                         start=True, stop=True)
                        nc.tensor.matmul(pgx[:, SC:2 * SC], ones_row[:, :], ninv[:, :],
                                         start=True, stop=True)
                        for dc in range(NDC):
                            t1 = x2p.tile([128, SC], BF16, tag="t1",
                                          name=f"t1_{name}_{sc}_{dc}", bufs=3)
                            nc.vector.tensor_mul(t1[:, :], prevT[:, dc, ts(sc, SC)],
                                                 pgx[:, 0:SC])
                            t2 = x2p.tile([128, SC], BF16, tag="t2",
                                          name=f"t2_{name}_{sc}_{dc}", bufs=3)
                            nc.vector.tensor_add(t2[:, :], t1[:, :], pgx[:, SC:2 * SC])
                            nc.scalar.activation(newT[:, dc, ts(sc, SC)], t2[:, :],
                                                 AF.Identity,
                                                 scale=lng_sb[:, dc:dc + 1],
                                                 bias=lnb_sb[:, dc:dc + 1])
                    return newT

                def attn_addin(mi):
                    def fn(sc, dc):
                        return (a_out[mi][sc][ts(dc, 128), :], None)
                    return fn

                with tc.tile_pool(name="attn", bufs=1) as attnp:
                    mask_sb = attnp.tile([128, 4 * SC], BF16, tag="mask")
                    nc.sync.dma_start(mask_sb[:, :], maskT[:, :])
                    qkw_sbs, vw_sbs = [], []
                    for mi in range(2):
                        qs = attnp.tile([128, NDC, 256], BF16, tag=f"qkw{mi}",
                                        name=f"qkw_sb{mi}")
                        nc.sync.dma_start(qs[:, :, :], qkw[mi][:, :, :])
                        vs = attnp.tile([128, NDC, 130], BF16, tag=f"vw{mi}",
                                        name=f"vw_sb{mi}")
                        nc.sync.dma_start(vs[:, :, :], vw[mi][:, :, :])
                        qkw_sbs.append(qs)
                        vw_sbs.append(vs)
                    mha_block(0, hT, True, attnp, qkw_sbs[0], vw_sbs[0])
                    h1T = ln_block(hT, attn_addin(0), "h1T")
                    mha_block(1, h1T, False, attnp, qkw_sbs[1], vw_sbs[1])

                # ---------- FFN (DFF sharded), per s-chunk, chunked bf16 AR --
                with tc.tile_pool(name="ffw", bufs=1) as ffp:
                    f1w_sb = ffp.tile([128, NDC, FS], BF16, tag="f1w")
                    nc.sync.dma_start(f1w_sb[:, :, :], f1w[:, :, :])
                    f2w_sb = ffp.tile([128, NFC, D], BF16, tag="f2w")
                    nc.sync.dma_start(f2w_sb[:, :, :], f2w[:, :, :])
                    h2T = ln_block(h1T, attn_addin(1), "h2T")
                    for sc in range(NSC):
                        uT = ffp.tile([128, NFC, SC], BF16, tag="uT", bufs=2,
                                      name=f"uT_{sc}")
                        for fc in range(NFC):
                            pu = psa.tile([128, SC], F32, tag="ps1", name=f"pu_{fc}_{sc}")
                            for dc in range(NDC):
                                nc.tensor.matmul(pu[:, :], f1w_sb[:, dc, ts(fc, 128)],
                                                 h2T[:, dc, ts(sc, SC)],
                                                 start=(dc == 0), stop=(dc == NDC - 1))
                            nc.scalar.activation(uT[:, fc, :], pu[:, :], AF.Relu,
                                                 bias=f1b_sb[:, fc:fc + 1])
                        for dc in range(NDC):
                            py = psa.tile([128, SC], F32, tag="ps1", name=f"py_{dc}_{sc}")
                            for fc in range(NFC):
                                nc.tensor.matmul(py[:, :], f2w_sb[:, fc, ts(dc, 128)],
                                                 uT[:, fc, :],
                                                 start=(fc == 0), stop=(fc == NFC - 1))
                            yv = evp.tile([128, SC], BF16, tag="yv", bufs=3,
                                          name=f"yv_{dc}_{sc}")
                            nc.scalar.activation(yv[:, :], py[:, :], AF.Copy)
                            nc.sync.dma_start(
                                y_par[sc // 2][ts(dc, 128), ts(sc % 2, SC)], yv[:, :])
                        if sc % 2 == 1:
                            nc.gpsimd.collective_compute(
                                "AllReduce", ALU.add, replica_groups=RG,
                                ins=[y_par[sc // 2][:, :].opt()],
                                outs=[y_red[sc // 2][:, :].opt()],
                            )

                def y_addin(sc, dc):
                    return (y_red[sc // 2][ts(dc, 128), ts(sc % 2, SC)],
                            f2bT_sb[:, dc:dc + 1])

                wvts = {}
                for vc in range(3):
                    wvt = owp.tile([128, NDC, 128], BF16, tag="ow", name=f"ow_{vc}")
                    nc.sync.dma_start(wvt[:, :, :], outw[vc, :, :, :])
                    wvts[vc] = wvt

                outT = ln_block(h2T, y_addin, "outT", dst_pool=outp, dst_tag="outT",
                                ad_eng=nc.scalar)

            if debug:
                for nm, t in (("hT", hT), ("h1T", h1T), ("h2T", h2T), ("outT", outT)):
                    for dc in range(NDC):
                        nc.sync.dma_start(dbg[nm][ts(dc, 128), :], t[:, dc, :])
                for sc in range(NSC):
                    nc.sync.dma_start(dbg["yr"][:, ts(sc, SC)], y_red[sc][:, :])
                    nc.sync.dma_start(dbg["a0"][:, ts(sc, SC)], a_out[0][sc][:, :])
            acts_ctx.__exit__(None, None, None)

            # ======== phase O: output GEMM + softmax over vocab ========
            # PSUM budget: pl([128,1024] x2 = 4 banks) + pss(4 banks)
            with tc.tile_pool(name="exp", bufs=NVC) as expp, \
                 tc.tile_pool(name="pp", bufs=2) as ppp, \
                 tc.tile_pool(name="osc", bufs=1) as osc, \
                 tc.tile_pool(name="ps_l", bufs=2, space="PSUM") as psl, \
                 tc.tile_pool(name="ps_s", bufs=1, space="PSUM") as pssp:
                pss = pssp.tile([65, S], F32, tag="pss")
                recb_sb = osc.tile([128, S], F32, tag="recb")
                ets = []
                for vc in range(NVC):
                    et = expp.tile([128, S], BF16, tag="eo", name=f"eo_{vc}")
                    ets.append(et)

                def gemm_half(hf, inject=None):
                    for vc in range(NVC):
                        if inject is not None and vc == 12:
                            inject()
                        if hf == 0 and vc in wvts:
                            wvt = wvts[vc]
                        else:
                            wvt = owp.tile([128, NDC, 128], BF16, tag="ow",
                                           name=f"ow_{hf}_{vc}")
                            nc.sync.dma_start(wvt[:, :, :], outw[vc, :, :, :])
                        pl = psl.tile([128, 2 * SC], F32, tag="pl",
                                      name=f"pl_{vc}_{hf}")
                        for sch in range(2):
                            sc = 2 * hf + sch
                            for dc in range(NDC):
                                nc.tensor.matmul(pl[:, ts(sch, SC)], wvt[:, dc, :],
                                                 outT[:, dc, ts(sc, SC)],
                                                 start=(dc == 0), stop=(dc == NDC - 1))
                        nc.scalar.activation(ets[vc][:, ts(hf, 2 * SC)], pl[:, :],
                                             AF.Exp, bias=outb_sb[:, vc:vc + 1])
                        for sch in range(2):
                            sc = 2 * hf + sch
                            nc.tensor.matmul(pss[0:1, ts(sc, SC)], ones_bf_col[:, :],
                                             ets[vc][:, ts(sc, SC)],
                                             start=(vc == 0), stop=(vc == NVC - 1))
                    hsl = slice(hf * 2 * SC, (hf + 1) * 2 * SC)
                    nc.scalar.activation(recb_sb[0:1, hsl], pss[0:1, hsl], AF.Copy)
                    nc.sync.dma_start(ss_in[hf][0:1, :], recb_sb[0:1, hsl])
                    nc.gpsimd.collective_compute(
                        "AllReduce", ALU.add, replica_groups=RG,
                        ins=[ss_in[hf][:, :].opt()], outs=[ss_out[hf][:, :].opt()],
                    )

                def tail_half(hf):
                    hsl = slice(hf * 2 * SC, (hf + 1) * 2 * SC)
                    rr = osc.tile([1, 2 * SC], F32, tag="rr", bufs=2, name=f"rr{hf}")
                    nc.scalar.dma_start(rr[:, :], ss_out[hf][0:1, :])
                    ri = osc.tile([1, 2 * SC], F32, tag="ri", bufs=2, name=f"ri{hf}")
                    nc.vector.reciprocal_approx_fast(ri[:, :], rr[:, :])
                    recb = psl.tile([128, 2 * SC], F32, tag="pl", name=f"recb{hf}")
                    for sch in range(2):
                        nc.tensor.matmul(recb[:, ts(sch, SC)], ones_row[:, :],
                                         ri[0:1, ts(sch, SC)], start=True, stop=True)
                    nc.scalar.activation(recb_sb[:, hsl], recb[:, :], AF.Copy)
                    for vc in range(NVC):
                        eng = nc.vector if vc % 2 == 0 else nc.gpsimd
                        deng = nc.sync if vc % 2 == 0 else nc.scalar
                        pp = ppp.tile([128, 2 * SC], F16,
                                      tag="ppv" if eng is nc.vector else "ppg",
                                      name=f"pp_{vc}_{hf}")
                        eng.tensor_mul(pp[:, :], ets[vc][:, hsl], recb_sb[:, hsl])
                        deng.dma_start(probsT[ts(vc, 128), hsl], pp[:, :])

                gemm_half(0)
                gemm_half(1, inject=lambda: tail_half(0))
                tail_half(1)

    nc.compile()
    return nc


def _positional_encoding():
    pos = np.arange(S, dtype=np.float32)[:, None]
    i = np.arange(0, D, 2, dtype=np.float32)
    ang = (pos * np.exp((-np.log(10000.0) * i / D).astype(np.float32))).astype(np.float32)
    pe = np.zeros((S, D), np.float32)
    pe[:, 0::2] = np.sin(ang)
    pe[:, 1::2] = np.cos(ang)
    return pe


def _bf(x):
    return np.ascontiguousarray(x).astype(ml_dtypes.bfloat16)


def _f32(x):
    return np.ascontiguousarray(x, dtype=np.float32)


def prepare_inputs(inp):
    """Full fp32 inputs -> per-core input maps (host-side sharding/layout)."""
    li = L - 1
    xT_full = np.ascontiguousarray(inp["x"].T)          # [V, S]
    peb = (inp["emb_b"][None, :] + _positional_encoding()).astype(np.float32)
    pebT = _bf(peb.T)                                    # [D, S] bf16

    # causal mask patterns for the 4 diagonal t-tiles of an s-chunk
    t_loc = np.arange(128)[:, None]
    s_loc = np.arange(SC)[None, :]
    maskT = np.concatenate(
        [((p * 128 + t_loc) <= s_loc).astype(np.float32) for p in range(4)], axis=1
    )
    maskT = _bf(maskT)                                   # [128, 2048]

    lngT = _f32(inp["ln_g"].reshape(NDC, 128).T)
    lnbT = _f32(inp["ln_b"].reshape(NDC, 128).T)
    f2bT = _f32(inp["ff_b2"][li].reshape(NDC, 128).T)

    in_maps = []
    for c in range(NCORES):
        m = {}
        xs = xT_full[c * VSR:(c + 1) * VSR]              # [4000, S]
        m["xT"] = _bf(np.concatenate([xs, np.zeros((VSP - VSR, S), np.float32)], 0))
        ew = inp["emb_W"][c * VSR:(c + 1) * VSR]
        m["embW"] = _bf(np.concatenate([ew, np.zeros((VSP - VSR, D), np.float32)], 0))
        m["pebT"] = pebT
        m["maskT"] = maskT
        for mi, (Wq, bq, Wk, bk, Wv, bvv) in enumerate([
            (inp["Wq1"][li], inp["bq1"][li], inp["Wk1"][li], inp["bk1"][li],
             inp["Wv1"][li], inp["bv1"][li]),
            (inp["Wq2"][li], inp["bq2"][li], inp["Wk2"][li], inp["bk2"][li],
             inp["Wv2"][li], inp["bv2"][li]),
        ]):
            h0, h1 = 2 * c, 2 * c + 1
            qk = np.concatenate([Wq[h0], Wq[h1], Wk[h0], Wk[h1]], axis=1)  # [D, 256]
            m[f"qkw{mi+1}"] = _bf(qk.reshape(NDC, 128, 256).transpose(1, 0, 2))
            m[f"bqk{mi+1}"] = _f32(np.stack(
                [np.concatenate([bq[h0], bq[h1]]),
                 np.concatenate([bk[h0], bk[h1]])], axis=1))
            vp = np.zeros((D, 130), np.float32)
            vp[:, 0:64] = Wv[h0]
            vp[:, 65:129] = Wv[h1]
            m[f"vw{mi+1}"] = _bf(vp.reshape(NDC, 128, 130).transpose(1, 0, 2))
            m[f"bv{mi+1}"] = _f32(np.concatenate([bvv[h0], bvv[h1]])[:, None])
        w1 = inp["ff_W1"][li][:, c * FS:(c + 1) * FS]    # [D, FS]
        m["f1w"] = _bf(w1.reshape(NDC, 128, FS).transpose(1, 0, 2))
        m["f1b"] = _f32(inp["ff_b1"][li][c * FS:(c + 1) * FS].reshape(NFC, 128).T)
        w2 = inp["ff_W2"][li][c * FS:(c + 1) * FS]       # [FS, D]
        m["f2w"] = _bf(w2.reshape(NFC, 128, D).transpose(1, 0, 2))
        m["f2bT"] = f2bT
        m["lngT"] = lngT
        m["lnbT"] = lnbT
        ow = inp["out_W"][:, c * VSR:(c + 1) * VSR]      # [D, 4000]
        ow = np.concatenate([ow, np.zeros((D, VSP - VSR), np.float32)], axis=1)
        m["outw"] = _bf(ow.reshape(NDC, 128, NVC, 128).transpose(2, 1, 0, 3))
        ob = np.full(VSP, -30.0, np.float32)
        ob[:VSR] = inp["out_b"][c * VSR:(c + 1) * VSR]
        m["outb"] = _f32(ob.reshape(NVC, 128).T)
        in_maps.append(m)
    return in_maps


_NC_CACHE = {}


def kernel(**inputs):
    import os
    inputs = {k: np.asarray(v, dtype=np.float32) for k, v in inputs.items()}
    debug = bool(int(os.environ.get("KB_DEBUG", "0")))
    key = f"nc{int(debug)}"
    if key not in _NC_CACHE:
        _NC_CACHE[key] = build_bass(debug=debug)
    nc = _NC_CACHE[key]
    in_maps = prepare_inputs(inputs)
    import os
    trace = bool(int(os.environ.get("KB_TRACE", "0")))
    tmpdir = os.environ.get("KB_TMPDIR") or None
    res = run_bass_kernel_spmd(nc, in_maps, list(range(NCORES)), trace=trace,
                               tmpdir=tmpdir)
    LAST_RESULTS["res"] = res
    shards = [np.asarray(res.results[c]["probsT"][:VSR], dtype=np.float32)
              for c in range(NCORES)]
    return np.ascontiguousarray(np.concatenate(shards, axis=0).T)
